# revision 30
# baseline (speedup 1.0000x reference)
"""Trainium2 Bass kernel for AbsoluteSinusoidal2DPE logits.

Math (flattened, N = H*W = 1024, D = 512):
    logits[b] = q[b] @ e^T + e @ (k[b] + e)^T          # [N, N] per batch

Key structure: e[(i,j)] = eh[i] + ew[j] (a 2D sinusoidal PE table is the
outer SUM of two 1D tables), recoverable exactly from the embed input as
eh'[i] = embed[i,0], ew'[j] = embed[0,j] - embed[0,0]. This makes both
einsums rank-64 instead of rank-512x1024:

    q @ e^T        = A (.) B      A = q @ eh^T, B = q @ ew^T   [N, 32]
    e @ (k+e)^T    = C (.) D      [C;D] = [eh;ew] @ k^T + CE   [32, N]
    CE             = [eh;ew] @ e^T  (batch-independent const)

and the [N, N] assembly out[r, (k,l)] = A[r,k] + B[r,l] + C[i(r)] + D[j(r)]
is ONE K=128 matmul per 128-row tile:  out_tile = ZT^T @ WF with
ZT = [A^T; B^T; onehot_i; onehot_j] and WF = [Ph; Pw; C; D] (Ph/Pw the
0/1 column-selector patterns, identical to the onehot rows).

Per core (2 batches): 36 matmuls/batch (16K streamed cols vs 65K for the
dense form), ~8.5 MB total DMA (fp16 in, fp16 out; the error margin is
huge because every output element carries a ~300-1100 magnitude e.e
offset -- measured rel err ~1e-3 vs the 2e-2 gate).

Fallback: if embed is not decomposable (residual check) or inputs exceed
fp16 range, use the dense fp32r program (the previous known-good kernel).
"""

import numpy as np

B, H, W, D = 16, 32, 32, 512
N = H * W            # 1024
NCORES = 8
BPC = B // NCORES    # batches per core
P = 128              # partitions
KO = D // P          # 4 contraction chunks
NT = N // P          # 8 output row tiles
MH = N // 512        # 2 output column halves (PSUM bank = 512 fp32)

_PROG = None   # cached low-rank program
_PROG_FB = None  # cached fallback (dense fp32r) program


# --------------------------------------------------------------- helpers ---

def _round_fp32r(x: np.ndarray) -> np.ndarray:
    """Round fp32 -> fp32r (RNE to 11 explicit mantissa bits)."""
    xi = x.view(np.uint32).astype(np.uint64)
    add = ((xi >> 12) & 1) + 0x7FF
    xi = (xi + add) & 0xFFFFF000
    return np.ascontiguousarray(xi.astype(np.uint32).view(np.float32))


def _onehot_const() -> np.ndarray:
    """[64, N] fp16: rows c<32 = delta(pos//32==c), rows 32+c = delta(pos%32==c)."""
    pos = np.arange(N)
    oh = np.zeros((64, N), dtype=np.float16)
    oh[pos // 32, pos] = 1.0
    oh[32 + pos % 32, pos] = 1.0
    return oh


# --------------------------------------------------------- low-rank path ---

def _build_program(n_batches: int = BPC, loop_reps: int = 0,
                   stages: str = "full"):
    """n_batches > BPC repeats the batch loop (cycling the same DRAM data);
    loop_reps > 0 wraps the whole body in a For_i hardware loop (timing
    instrument); stages in {"load","rank","drain","full"} strips trailing
    pipeline stages (timing decomposition). The real kernel uses defaults."""
    import contextlib
    import concourse.mybir as mybir
    import concourse.tile as tile
    from concourse import bacc

    F32 = mybir.dt.float32
    F16 = mybir.dt.float16

    nc = bacc.Bacc()
    qt_d = nc.dram_tensor("qt", [BPC, D, N], F16, kind="ExternalInput")
    kt_d = nc.dram_tensor("kt", [BPC, D, N], F16, kind="ExternalInput")
    ehwt_d = nc.dram_tensor("ehwt", [D, 64], F16, kind="ExternalInput")
    oneh_d = nc.dram_tensor("oneh", [64, N], F16, kind="ExternalInput")
    ce_d = nc.dram_tensor("ce", [64, N], F16, kind="ExternalInput")
    eye_d = nc.dram_tensor("eye", [64, 64], F16, kind="ExternalInput")
    out_d = nc.dram_tensor("out", [BPC, N, N], F16, kind="ExternalOutput")

    with tile.TileContext(nc) as tc:
        with (
            tc.tile_pool(name="cst", bufs=1) as cst,
            tc.tile_pool(name="inp", bufs=2) as inp,
            tc.tile_pool(name="zw", bufs=1) as zw,
            tc.tile_pool(name="outp", bufs=4) as outp,
            tc.tile_pool(name="psA", bufs=2, space="PSUM") as psA,
            tc.tile_pool(name="psF", bufs=2, space="PSUM") as psF,
        ):
          loop_cm = tc.For_i(0, loop_reps, 1) if loop_reps else contextlib.nullcontext()
          with loop_cm:
            ehwt = cst.tile([P, KO, 64], F16, name="ehwt")
            oneh = cst.tile([64, N], F16, name="oneh")
            ce = cst.tile([64, N], F16, name="ce")
            eye = cst.tile([64, 64], F16, name="eye")

            # PE pre-warm: dummy matmuls while the first input DMAs are in
            # flight so the HAM clock gate is open when real work starts
            warm = cst.tile([P, 512], F16, name="warm")
            nc.vector.memset(warm[:].bitcast(F32), 0.0)
            for w in range(12):
                warm_ps = psF.tile([P, 512], F32, tag="psf", name="warm_ps")
                nc.tensor.matmul(warm_ps[:], warm[:, 0:128], warm[:],
                                 start=True, stop=True)

            # zt/wf double buffers: zt = [A^T; B^T; onehot_i; onehot_j],
            # wf = [Ph; Pw; C; D]. One-hot rows written once per buffer.
            zts = [zw.tile([P, N], F16, tag=f"zt{i}", name=f"zt{i}")
                   for i in range(2)]
            wfs = [zw.tile([P, N], F16, tag=f"wf{i}", name=f"wf{i}")
                   for i in range(2)]
            inited = [False, False]

            def load_inputs(b):
                """column-half input DMAs; each half unlocks that half of
                ABT/CDT (the contraction dim is fully present per half)"""
                bi = b % BPC
                qt = inp.tile([P, KO, N], F16, tag="qt", name="qt")
                kt = inp.tile([P, KO, N], F16, tag="kt", name="kt")
                qt_src = qt_d[bi].rearrange("(ko p) m -> p ko m", p=P)
                kt_src = kt_d[bi].rearrange("(ko p) m -> p ko m", p=P)
                if b % 2 == 0:
                    # first batch of a pair: monolithic 1MB loads (measured
                    # ~580 GB/s vs ~400 for 512KB chunks)
                    nc.sync.dma_start(qt[:], qt_src)
                    if b == 0:
                        # consts on the scalar ring: store-free at this
                        # point, so they stream beside the input ring
                        nc.scalar.dma_start(
                            ehwt[:],
                            ehwt_d.rearrange("(ko p) m -> p ko m", p=P))
                        nc.scalar.dma_start(ce[:], ce_d[:, :])
                        nc.scalar.dma_start(eye[:], eye_d[:, :])
                        nc.scalar.dma_start(oneh[:], oneh_d[:, :])
                    nc.sync.dma_start(kt[:], kt_src)
                else:
                    # last batch of the pair: column-half loads so the tail
                    # chain (pc-h1 -> wf-h1 -> nh1 finals) starts before the
                    # final 512KB lands
                    for h in range(2):
                        hs = slice(h * 512, (h + 1) * 512)
                        nc.sync.dma_start(qt[:, :, hs], qt_src[:, :, hs])
                        nc.sync.dma_start(kt[:, :, hs], kt_src[:, :, hs])
                return qt, kt

            def rank64(b, qt, kt):
                """ABT+CDT packed into ONE [128, 512] PSUM bank per n-half:
                ABT -> partitions 0-63 (col group 0), CDT -> partitions
                64-127 (col group 64 via tile_position). A single start=True
                (the first ABT matmul) clears the bank's has_written bits;
                every later matmul overwrites-where-clear / accumulates-
                where-set, so the two logical groups coexist in the bank.
                Half the banks -> double-buffered across batches (no
                serialization of batch b+1's matmuls on batch b's copies)."""
                ph = psA.tile([P, N], F32, tag="pab", name="pab")
                for nh in range(MH):
                    ns = slice(nh * 512, (nh + 1) * 512)
                    for ko in range(KO):
                        nc.tensor.matmul(ph[0:64, ns], ehwt[:, ko],
                                         qt[:, ko, ns], start=(ko == 0),
                                         stop=False, skip_group_check=True)
                    nc.tensor.matmul(ph[64:128, ns], eye[:], ce[:, ns],
                                     start=True, stop=False,
                                     tile_position=(0, 64),
                                     skip_group_check=True)
                    for ko in range(KO):
                        nc.tensor.matmul(ph[64:128, ns], ehwt[:, ko],
                                         kt[:, ko, ns], start=False,
                                         stop=(ko == KO - 1),
                                         tile_position=(0, 64),
                                         skip_group_check=True)
                return ph

            def build_ztwf(b, pab):
                """per-half PSUM->SBUF f16 copies: zt halves on ACT, wf halves
                on DVE (plus one-time one-hot inits per buffer on DVE)"""
                i = b % 2
                if not inited[i]:
                    nc.vector.tensor_copy(zts[i][64:128, :], oneh[:])
                    nc.vector.tensor_copy(wfs[i][0:64, :], oneh[:])
                    inited[i] = True
                nc.scalar.copy(zts[i][0:64, :], pab[0:64, :])
                nc.vector.tensor_copy(wfs[i][64:128, :], pab[64:128, :])

            def finals(b, nts, tail=False):
                bi = b % BPC
                zt, wf = zts[b % 2], wfs[b % 2]
                # stores merged up to 4 row-tiles per DMA (1MB ~ line rate),
                # shrinking toward the kernel tail so the last store's drain
                # chain is short; all stores OFF the sync ring so loads never
                # queue behind them
                out_rows = out_d[bi].rearrange("(nt p) m -> nt p m", p=P)
                groups = [(nts[0], 4)] if len(nts) == 4 else [(0, 4), (4, 2), (6, 1), (7, 1)]
                if not tail and len(nts) == 8:
                    groups = [(0, 4), (4, 4)]
                for g0, glen in groups:
                    ob = outp.tile([P, glen, N], F16, tag=f"ob{glen}",
                                   name="ob")
                    for j in range(glen):
                        nt = g0 + j
                        if tail and nt % 2 == 0:
                            # the last batch's finals borrow the (now idle)
                            # pab slots: 4 two-bank slots in rotation
                            ps = psA.tile([P, N], F32, tag="pab", name="psx")
                        else:
                            ps = psF.tile([P, N], F32, tag="psf", name="psf")
                        for nh in range(MH):
                            ns = slice(nh * 512, (nh + 1) * 512)
                            nc.tensor.matmul(ps[:, ns],
                                             zt[:, nt * P:(nt + 1) * P],
                                             wf[:, ns], start=True, stop=True)
                        # one full-row-tile drain (halves op count + links)
                        if nt % 2 == 0:
                            nc.vector.tensor_copy(ob[:, j, :], ps[:])
                        else:
                            nc.scalar.copy(ob[:, j, :], ps[:])
                    if stages == "drain":
                        continue
                    dst = out_rows[g0:g0 + glen].rearrange("nt p m -> p nt m")
                    if tail and g0 + glen > nts[-1]:
                        # split the very last store across both rings so its
                        # HBM receipt overlaps the other half's stream
                        nc.scalar.dma_start(out_rows[g0][:, 0:512],
                                            ob[:, 0, 0:512])
                        nc.sync.dma_start(out_rows[g0][:, 512:N],
                                          ob[:, 0, 512:N])
                    elif (g0 // 4 + b) % 2 == 0:
                        nc.scalar.dma_start(dst, ob[:])
                    else:
                        nc.sync.dma_start(dst, ob[:])

            # batches in pairs; emission interleaved so the second batch's
            # ZT/WF copies sit between the first batch's drain groups in the
            # DVE/ACT FIFOs (no engine blocks on a not-yet-ready copy)
            for pair0 in range(0, n_batches, 2):
                b0, b1 = pair0, pair0 + 1
                have_b1 = b1 < n_batches
                qt0, kt0 = load_inputs(b0)
                if have_b1:
                    qt1, kt1 = load_inputs(b1)
                if stages == "load":
                    continue
                pab0 = rank64(b0, qt0, kt0)
                build_ztwf(b0, pab0)
                if have_b1:
                    pab1 = rank64(b1, qt1, kt1)
                if stages == "rank":
                    if have_b1:
                        build_ztwf(b1, pab1)
                    continue
                finals(b0, [0, 1, 2, 3], tail=not have_b1 and False)
                if have_b1:
                    build_ztwf(b1, pab1)
                last = pair0 + 2 >= n_batches
                finals(b0, [4, 5, 6, 7], tail=last and not have_b1)
                if have_b1:
                    finals(b1, list(range(NT)), tail=last)

    nc.compile()
    return nc


# --------------------------------------------------- dense fp32r fallback ---

def _build_program_fallback(n_batches: int = BPC):
    """Dense fp32r program (previous known-good kernel), used only if embed
    is not decomposable as eh[i]+ew[j] or inputs exceed fp16 range."""
    import concourse.mybir as mybir
    import concourse.tile as tile
    from concourse import bacc

    F32 = mybir.dt.float32
    F32R = mybir.dt.float32r

    nc = bacc.Bacc()
    qt_d = nc.dram_tensor("qt", [BPC, D, N], F32R, kind="ExternalInput")
    kt_d = nc.dram_tensor("kt", [BPC, D, N], F32, kind="ExternalInput")
    et_d = nc.dram_tensor("et", [D, N], F32R, kind="ExternalInput")
    out_d = nc.dram_tensor("out", [BPC, N, N], F32, kind="ExternalOutput")

    with tile.TileContext(nc) as tc:
        with (
            tc.tile_pool(name="etp", bufs=1) as etp,
            tc.tile_pool(name="inp", bufs=2) as inp,
            tc.tile_pool(name="outp", bufs=8) as outp,
            tc.tile_pool(name="ps", bufs=1, space="PSUM") as psp,
        ):
            et = etp.tile([P, KO, N], F32R, name="et")
            et_src = et_d.rearrange("(ko p) m -> p ko m", p=P)
            for ko in range(KO):
                nc.sync.dma_start(et[:, ko], et_src[:, ko])
            for b in range(n_batches):
                bi = b % BPC
                qt = inp.tile([P, KO, N], F32R, tag="qt")
                kt = inp.tile([P, KO, N], F32, tag="kt")
                kpe = inp.tile([P, KO, N], F32R, tag="kpe")
                qt_src = qt_d[bi].rearrange("(ko p) m -> p ko m", p=P)
                kt_src = kt_d[bi].rearrange("(ko p) m -> p ko m", p=P)
                for ko in range(KO):
                    nc.sync.dma_start(qt[:, ko], qt_src[:, ko])
                for ko in range(KO):
                    nc.sync.dma_start(kt[:, ko], kt_src[:, ko])
                    nc.vector.tensor_add(kpe[:, ko], kt[:, ko],
                                         et[:, ko].bitcast(F32))
                out_rows = out_d[bi].rearrange("(nt p) m -> nt p m", p=P)
                for mh in range(MH):
                    ms = slice(mh * 512, (mh + 1) * 512)
                    for nt in range(NT):
                        ps = psp.tile([P, 512], F32, tag=f"ps{nt}",
                                      name=f"ps{nt}")
                        for ko in range(KO):
                            nc.tensor.matmul(
                                ps[:], qt[:, ko, nt * P:(nt + 1) * P],
                                et[:, ko, ms], start=(ko == 0), stop=False)
                        for ko in range(KO):
                            nc.tensor.matmul(
                                ps[:], et[:, ko, nt * P:(nt + 1) * P],
                                kpe[:, ko, ms], start=False, stop=(ko == KO - 1))
                        ob = outp.tile([P, 512], F32, tag="ob")
                        nc.vector.tensor_copy(ob[:], ps[:])
                        nc.scalar.dma_start(out_rows[nt][:, ms], ob[:])

    nc.compile()
    return nc


def _kernel_fallback(qf, kf, ef):
    global _PROG_FB
    from concourse import bass_utils

    qt = _round_fp32r(np.ascontiguousarray(qf.transpose(0, 2, 1)))
    kt = np.ascontiguousarray(kf.transpose(0, 2, 1))
    et = _round_fp32r(np.ascontiguousarray(ef.T))

    if _PROG_FB is None:
        _PROG_FB = _build_program_fallback()
    in_maps = []
    for c in range(NCORES):
        sl = slice(c * BPC, (c + 1) * BPC)
        in_maps.append({"qt": qt[sl], "kt": kt[sl], "et": et})
    res = bass_utils.run_bass_kernel_spmd(_PROG_FB, in_maps,
                                          core_ids=list(range(NCORES)))
    outs = [r["out"] for r in res.results]
    return np.concatenate(outs, axis=0)


# ------------------------------------------------------------------ entry ---

def kernel(q: np.ndarray, k: np.ndarray, embed: np.ndarray) -> np.ndarray:
    global _PROG
    from concourse import bass_utils

    q = np.asarray(q)
    k = np.asarray(k)
    embed = np.asarray(embed)
    assert q.shape == (B, H, W, D) and k.shape == (B, H, W, D)
    assert embed.shape == (H, W, D)

    qf = q.reshape(B, N, D).astype(np.float32, copy=False)
    kf = k.reshape(B, N, D).astype(np.float32, copy=False)
    ef = embed.reshape(N, D).astype(np.float32, copy=False)

    # embed decomposition: eh'[i] = embed[i,0], ew'[j] = embed[0,j]-embed[0,0]
    ehp = embed[:, 0, :].astype(np.float32)
    ewp = (embed[0, :, :] - embed[0, 0, :]).astype(np.float32)
    recon = ehp[:, None, :] + ewp[None, :, :]
    resid = np.abs(embed.astype(np.float32) - recon).max()
    in_absmax = max(np.abs(qf).max(), np.abs(kf).max(), np.abs(ef).max())
    if resid > 1e-3 or not np.isfinite(in_absmax) or in_absmax > 1e4:
        out = _kernel_fallback(qf, kf, ef)
        return np.ascontiguousarray(out.reshape(B, H, W, H, W))

    ehw = np.concatenate([ehp, ewp], axis=0)           # [64, D]
    ehw16 = ehw.astype(np.float16)
    ehwt = np.ascontiguousarray(ehw16.T)               # [D, 64] fp16
    ce = (ehw16.astype(np.float32) @ ef.T).astype(np.float16)  # [64, N]
    oneh = _onehot_const()
    eye = np.eye(64, dtype=np.float16)

    qt = np.ascontiguousarray(qf.transpose(0, 2, 1)).astype(np.float16)
    kt = np.ascontiguousarray(kf.transpose(0, 2, 1)).astype(np.float16)

    if _PROG is None:
        _PROG = _build_program()
    in_maps = []
    for c in range(NCORES):
        sl = slice(c * BPC, (c + 1) * BPC)
        in_maps.append({"qt": qt[sl], "kt": kt[sl], "ehwt": ehwt,
                        "oneh": oneh, "ce": ce, "eye": eye})
    res = bass_utils.run_bass_kernel_spmd(_PROG, in_maps,
                                          core_ids=list(range(NCORES)))
    outs = [r["out"] for r in res.results]              # each [BPC, N, N] f16
    full = np.concatenate(outs, axis=0).astype(np.float32)
    return np.ascontiguousarray(full.reshape(B, H, W, H, W))
